# revision 1
# baseline (speedup 1.0000x reference)
"""Trainium2 Bass kernel for nn_CustomModelEmbeddingBagGroup.

Math: the reference sums every bag then sums over bags, so the offsets
cancel out and the answer is

    out = sum_i r[eb_input[i]],   r = rowsum_D(5*W0 + 10*W1 + 6*W2)

Sharding (8 cores): tables are sharded row-wise (vocab dim); indices are
routed to the owning shard on the host (the "all-to-all" of the
model-parallel embedding recipe), the final scalar reduce happens on the
host over the 8 per-core partials.

Per-core device program:
  pass 1: stream the 3 table shards, compute wc = 5*w0+10*w1+6*w2 (bf16)
          back to DRAM and r = rowsum(wc) in SBUF.
  pass 2: dma_gather wc rows for the core's indices, reduce with a
          ones-vector matmul on the tensor engine into PSUM, then to a
          [1,1] partial.
"""

import numpy as np

import concourse.bass as bass
import concourse.bacc as bacc
import concourse.mybir as mybir
import concourse.tile as tile
from concourse.bass_utils import run_bass_kernel_spmd

F32 = mybir.dt.float32
BF16 = mybir.dt.bfloat16
I16 = mybir.dt.int16

NCORES = 8
V = 100000
D = 128
SH = 12544          # vocab rows owned per core (8*12544 = 100352 >= V)
VROWS = 12672       # 99*128; rows >= SH are zero padding (pad index target)
PAD_IDX = 12544     # local index used for padding; row is all zeros
NIDX = 106496       # 832*128 = 13*8192 per-core index capacity
CHUNK = 8192        # indices per dma_gather call
RT = 11             # 128-row chunks per pass-1 tile (99 = 9*11)
NT = 9


def build_nc(loop=1, stage=2):
    nc = bacc.Bacc("TRN2", target_bir_lowering=False, debug=False,
                   num_devices=NCORES)
    w0 = nc.dram_tensor("w0", [VROWS, D], F32, kind="ExternalInput")
    w1 = nc.dram_tensor("w1", [VROWS, D], F32, kind="ExternalInput")
    w2 = nc.dram_tensor("w2", [VROWS, D], F32, kind="ExternalInput")
    idx = nc.dram_tensor("idx", [128, NIDX // 16], I16, kind="ExternalInput")
    out = nc.dram_tensor("out", [1, 1], F32, kind="ExternalOutput")

    with tile.TileContext(nc) as tc:
        with (
            tc.tile_pool(name="sbuf", bufs=2) as pool,
            tc.tile_pool(name="gat", bufs=3) as gpool,
            tc.tile_pool(name="const", bufs=1) as cpool,
            tc.tile_pool(name="dram", bufs=1, space="DRAM") as dpool,
            tc.tile_pool(name="psum", bufs=1, space="PSUM") as ppool,
        ):
            wc_dram = dpool.tile([VROWS, D], BF16)
            ones = cpool.tile([128, 1], BF16)
            nc.vector.memset(ones[:], 1.0)
            idx_sb = cpool.tile([128, NIDX // 16], I16)
            nc.sync.dma_start(idx_sb[:], idx[:])
            r_sb = cpool.tile([128, NT * RT], F32)

            import contextlib
            loop_cm = tc.For_i(0, loop, 1) if loop > 1 else contextlib.nullcontext()
            with loop_cm:
                body(nc, tc, pool, gpool, cpool, dpool, ppool,
                     wc_dram, ones, idx_sb, r_sb, w0, w1, w2, out, stage)

    nc.finalize()
    return nc


def body(nc, tc, pool, gpool, cpool, dpool, ppool,
         wc_dram, ones, idx_sb, r_sb, w0, w1, w2, out, stage=2):
            w0v = w0[:].rearrange("(n p) d -> n p d", p=128)
            w1v = w1[:].rearrange("(n p) d -> n p d", p=128)
            w2v = w2[:].rearrange("(n p) d -> n p d", p=128)
            wcv = wc_dram[:].rearrange("(n p) d -> n p d", p=128)

            # pass 1: combine tables, write wc (bf16), accumulate rowsums
            for t in range(NT):
                sl = slice(t * RT, (t + 1) * RT)
                a0 = pool.tile([128, RT, D], F32, tag="a0")
                a1 = pool.tile([128, RT, D], F32, tag="a1")
                a2 = pool.tile([128, RT, D], F32, tag="a2")
                nc.sync.dma_start(a0[:], w0v[sl].rearrange("c p d -> p c d"))
                nc.sync.dma_start(a1[:], w1v[sl].rearrange("c p d -> p c d"))
                nc.sync.dma_start(a2[:], w2v[sl].rearrange("c p d -> p c d"))
                t2 = pool.tile([128, RT, D], F32, tag="t2")
                nc.scalar.mul(t2[:], a2[:], 6.0)
                acc = pool.tile([128, RT, D], F32, tag="acc")
                nc.vector.scalar_tensor_tensor(
                    acc[:], a0[:], 5.0, t2[:],
                    mybir.AluOpType.mult, mybir.AluOpType.add)
                wc_bf = pool.tile([128, RT, D], BF16, tag="wcbf")
                nc.vector.scalar_tensor_tensor(
                    wc_bf[:], a1[:], 10.0, acc[:],
                    mybir.AluOpType.mult, mybir.AluOpType.add)
                nc.sync.dma_start(wcv[sl].rearrange("c p d -> p c d"), wc_bf[:])
                nc.vector.tensor_reduce(
                    r_sb[:, sl], wc_bf[:], mybir.AxisListType.X,
                    mybir.AluOpType.add)

            # pass 2: gather rows of wc and sum everything
            nch = (NIDX // CHUNK) if stage >= 1 else 0
            if stage >= 2:
                psum = ppool.tile([1, 512], F32)
            for k in range(nch):
                g = gpool.tile([128, CHUNK // 128, D], BF16, tag="g")
                nc.gpsimd.dma_gather(
                    g[:], wc_dram[:],
                    idx_sb[:, k * (CHUNK // 16):(k + 1) * (CHUNK // 16)],
                    CHUNK, CHUNK, D, single_packet=False)
                if stage >= 2:
                    gv = g[:].rearrange("p c d -> p (c d)")
                    for m in range(CHUNK // 512):
                        nc.tensor.matmul(
                            psum[:], ones[:], gv[:, m * 512:(m + 1) * 512],
                            start=(k == 0 and m == 0),
                            stop=(k == nch - 1 and m == CHUNK // 512 - 1))

            res = cpool.tile([1, 1], F32)
            if stage >= 2:
                nc.vector.tensor_reduce(
                    res[:], psum[:], mybir.AxisListType.X, mybir.AluOpType.add)
            else:
                nc.vector.tensor_copy(res[:], r_sb[0:1, 0:1])
            nc.sync.dma_start(out[:], res[:])


_NC_CACHE = {}


def _get_nc(loop=1, stage=2):
    key = (loop, stage)
    if key not in _NC_CACHE:
        _NC_CACHE[key] = build_nc(loop, stage)
    return _NC_CACHE[key]


def _shard_inputs(eb_input, W0, W1, W2):
    """Route indices / slice tables per core. Pure data movement."""
    idx = np.asarray(eb_input).astype(np.int64, copy=False)
    in_maps = []
    order = np.argsort(idx // SH, kind="stable")
    sorted_idx = idx[order]
    bounds = np.searchsorted(sorted_idx, np.arange(NCORES + 1) * SH)
    for c in range(NCORES):
        lo, hi = c * SH, (c + 1) * SH
        li = (sorted_idx[bounds[c]:bounds[c + 1]] - lo).astype(np.int16)
        n = li.shape[0]
        if n > NIDX:
            raise ValueError(f"core {c} bucket {n} > NIDX {NIDX}")
        pad = np.full(NIDX, PAD_IDX, np.int16)
        pad[:n] = li
        idx16 = np.ascontiguousarray(np.tile(pad.reshape(-1, 16).T, (8, 1)))

        def shard(W):
            ws = np.zeros((VROWS, D), np.float32)
            real = min(hi, V) - lo
            if real > 0:
                ws[:real] = W[lo:lo + real]
            return ws

        in_maps.append({
            "w0": shard(W0), "w1": shard(W1), "w2": shard(W2),
            "idx": idx16,
        })
    return in_maps


def _run(inputs, trace=False):
    nc = _get_nc()
    in_maps = _shard_inputs(inputs["eb_input"], inputs["W0"], inputs["W1"],
                            inputs["W2"])
    res = run_bass_kernel_spmd(nc, in_maps, core_ids=list(range(NCORES)),
                               trace=trace)
    total = np.float64(0.0)
    for r in res.results:
        total += np.float64(r["out"][0, 0])
    return np.float32(total), res


def kernel(**inputs) -> np.ndarray:
    out, _ = _run(inputs, trace=False)
    return np.asarray(out)


def _bench(inputs, iters=30, warmup=5, loop=1, stage=2):
    """Steady-state timing: build the sharded jit once, keep inputs on
    device, time repeated executions. Returns (per_call_seconds_list, out)."""
    import time
    import jax
    from jax.experimental.shard_map import shard_map
    from jax.sharding import Mesh, PartitionSpec
    from concourse import bass2jax, mybir as _mb

    nc = _get_nc(loop, stage)
    in_maps = _shard_inputs(inputs["eb_input"], inputs["W0"], inputs["W1"],
                            inputs["W2"])
    bass2jax.install_neuronx_cc_hook()

    partition_name = (nc.partition_id_tensor.name
                      if nc.partition_id_tensor else None)
    in_names, out_names, out_avals, zero_outs = [], [], [], []
    for alloc in nc.m.functions[0].allocations:
        if not isinstance(alloc, _mb.MemoryLocationSet):
            continue
        name = alloc.memorylocations[0].name
        if alloc.kind == "ExternalInput":
            if name != partition_name:
                in_names.append(name)
        elif alloc.kind == "ExternalOutput":
            out_names.append(name)
            shape = tuple(alloc.tensor_shape)
            dtype = _mb.dt.np(alloc.dtype)
            out_avals.append(jax.core.ShapedArray(shape, dtype))
            zero_outs.append(np.zeros(shape, dtype))
    n_params = len(in_names)
    all_in_names = list(in_names) + list(out_names)
    if partition_name is not None:
        all_in_names.append(partition_name)

    def _body(*args):
        operands = list(args)
        if partition_name is not None:
            operands.append(bass2jax.partition_id_tensor())
        outs = bass2jax._bass_exec_p.bind(
            *operands,
            out_avals=tuple(out_avals),
            in_names=tuple(all_in_names),
            out_names=tuple(out_names),
            lowering_input_output_aliases=(),
            sim_require_finite=True,
            sim_require_nnan=True,
            nc=nc,
        )
        return tuple(outs)

    n_cores = NCORES
    devices = jax.devices()[:n_cores]
    mesh = Mesh(np.asarray(devices), ("core",))
    in_specs = (PartitionSpec("core"),) * (n_params + len(out_names))
    out_specs = (PartitionSpec("core"),) * len(out_names)
    donate = tuple(range(n_params, n_params + len(out_names)))
    sharded = jax.jit(shard_map(_body, mesh=mesh, in_specs=in_specs,
                                out_specs=out_specs, check_rep=False),
                      donate_argnums=donate, keep_unused=True)

    concat_in = [np.concatenate([in_maps[c][nm] for c in range(n_cores)],
                                axis=0) for nm in in_names]
    concat_zeros = [np.zeros((n_cores * z.shape[0], *z.shape[1:]), z.dtype)
                    for z in zero_outs]
    from jax.sharding import NamedSharding
    dev_in = [jax.device_put(a, NamedSharding(mesh, PartitionSpec("core")))
              for a in concat_in]
    out = None
    for _ in range(warmup):
        out = sharded(*dev_in, *concat_zeros)
        jax.block_until_ready(out)
    times = []
    for _ in range(iters):
        t0 = time.perf_counter()
        out = sharded(*dev_in, *concat_zeros)
        jax.block_until_ready(out)
        times.append(time.perf_counter() - t0)
    total = sum(np.float64(np.asarray(out[i]).reshape(n_cores, -1)[c, 0])
                for i, nm in enumerate(out_names) if nm == "out"
                for c in range(n_cores))
    return times, np.float32(total)



# revision 5
# speedup vs baseline: 2.4596x; 2.4596x over previous
"""Trainium2 Bass kernel for nn_CustomModelEmbeddingBagGroup.

Math: the reference sums every bag then sums over bags, so the offsets
cancel out and the answer is

    out = sum_i r[eb_input[i]],   r = rowsum_D(5*W0 + 10*W1 + 6*W2)

Sharding (8 cores): tables are sharded row-wise (vocab dim); indices are
routed to the owning shard on the host (the "all-to-all" of the
model-parallel embedding recipe), the final scalar reduce happens on the
host over the 8 per-core partials.

Per-core device program (v2 — no per-index DMA descriptors):
  pass 1: stream the 3 table shards, rowsum each tile on DVE,
          combine r = 5*r0 + 10*r1 + 6*r2  ([128, 98] f32).
  bcast:  bounce r through DRAM and broadcast the 12544-entry vector to
          all 128 partitions (r is laid out so DRAM order == vocab order).
  gather: ONE gpsimd.ap_gather — each of the 8 Q7 cores gathers scalars
          r[idx] for its own 13312 indices from SBUF (16 partitions in a
          group produce identical copies; divide by 16 at the end).
  reduce: rowsum the gathered [128, 13312] on DVE, partition_all_reduce
          on gpsimd, scale by 1/16, DMA the [1,1] partial out.
"""

import numpy as np

import concourse.bass as bass
import concourse.bass_isa as bass_isa
import concourse.bacc as bacc
import concourse.mybir as mybir
import concourse.tile as tile
from concourse.bass_utils import run_bass_kernel_spmd

F32 = mybir.dt.float32
BF16 = mybir.dt.bfloat16
I16 = mybir.dt.int16

NCORES = 8
V = 100000
D = 128
SH = 12544          # vocab rows owned per core = 98 * 128 (8*12544 >= V)
NCOL = 98           # free-dim columns of the rowsum accumulator
QCAP = 13312        # indices per Q7 DSP core (8 * 13312 = 106496 per core)
NIDX = QCAP * 8     # per-NeuronCore index capacity
PAD_IDX = SH        # gather slot holding 0.0 (r buffer has SH+1 slots)
NELEMS = SH + 1
NT = 14             # pass-1 tiles
RT = 7              # 128-row chunks per tile (14*7 = 98)


def build_nc(loop=1, stage=2):
    nc = bacc.Bacc("TRN2", target_bir_lowering=False, debug=False,
                   num_devices=NCORES)
    w0 = nc.dram_tensor("w0", [SH, D], F32, kind="ExternalInput")
    w1 = nc.dram_tensor("w1", [SH, D], F32, kind="ExternalInput")
    w2 = nc.dram_tensor("w2", [SH, D], F32, kind="ExternalInput")
    idx = nc.dram_tensor("idx", [128, QCAP // 16], I16, kind="ExternalInput")
    out = nc.dram_tensor("out", [1, 1], F32, kind="ExternalOutput")

    with tile.TileContext(nc) as tc:
        with (
            tc.tile_pool(name="sbuf", bufs=2) as pool,
            tc.tile_pool(name="big", bufs=1) as bpool,
            tc.tile_pool(name="const", bufs=1) as cpool,
            tc.tile_pool(name="dram", bufs=1, space="DRAM") as dpool,
        ):
            idx_sb = cpool.tile([128, QCAP // 16], I16)
            nc.sync.dma_start(idx_sb[:], idx[:])
            r_dram = dpool.tile([128, NCOL], F32)
            rrep = bpool.tile([128, NELEMS], F32)
            gout = bpool.tile([128, QCAP], F32)

            import contextlib
            loop_cm = tc.For_i(0, loop, 1) if loop > 1 else contextlib.nullcontext()
            with loop_cm:
                body(nc, tc, pool, bpool, cpool, dpool,
                     idx_sb, r_dram, rrep, gout, w0, w1, w2, out, stage)

    nc.finalize()
    return nc


def body(nc, tc, pool, bpool, cpool, dpool,
         idx_sb, r_dram, rrep, gout, w0, w1, w2, out, stage=2):
    w0v = w0[:].rearrange("(n p) d -> n p d", p=128)
    w1v = w1[:].rearrange("(n p) d -> n p d", p=128)
    w2v = w2[:].rearrange("(n p) d -> n p d", p=128)

    r0 = pool.tile([128, NCOL], F32, tag="r0")
    r1 = pool.tile([128, NCOL], F32, tag="r1")
    r2 = pool.tile([128, NCOL], F32, tag="r2")

    # pass 1: stream the 3 table shards, rowsum each tile
    for t in range(NT):
        sl = slice(t * RT, (t + 1) * RT)
        a0 = pool.tile([128, RT, D], F32, tag="a0")
        a1 = pool.tile([128, RT, D], F32, tag="a1")
        a2 = pool.tile([128, RT, D], F32, tag="a2")
        nc.sync.dma_start(a0[:], w0v[sl].rearrange("c p d -> p c d"))
        nc.sync.dma_start(a1[:], w1v[sl].rearrange("c p d -> p c d"))
        nc.sync.dma_start(a2[:], w2v[sl].rearrange("c p d -> p c d"))
        nc.vector.tensor_reduce(r0[:, sl], a0[:], mybir.AxisListType.X,
                                mybir.AluOpType.add)
        nc.vector.tensor_reduce(r1[:, sl], a1[:], mybir.AxisListType.X,
                                mybir.AluOpType.add)
        nc.vector.tensor_reduce(r2[:, sl], a2[:], mybir.AxisListType.X,
                                mybir.AluOpType.add)

    # r = 5*r0 + 10*r1 + 6*r2  (laid out so DRAM row-major order == vocab order)
    r6 = pool.tile([128, NCOL], F32, tag="r6")
    nc.scalar.mul(r6[:], r2[:], 6.0)
    racc = pool.tile([128, NCOL], F32, tag="racc")
    nc.vector.scalar_tensor_tensor(
        racc[:], r0[:], 5.0, r6[:],
        mybir.AluOpType.mult, mybir.AluOpType.add)
    r_sb = pool.tile([128, NCOL], F32, tag="r_sb")
    nc.vector.scalar_tensor_tensor(
        r_sb[:], r1[:], 10.0, racc[:],
        mybir.AluOpType.mult, mybir.AluOpType.add)

    # bounce through DRAM to broadcast r (12544 f32) to every partition
    nc.sync.dma_start(r_dram[:], r_sb[:])
    nc.sync.dma_start(
        rrep[:, 0:SH],
        r_dram[:].rearrange("p n -> (p n)").unsqueeze(0).partition_broadcast(128),
    )
    nc.vector.memset(rrep[:, SH:NELEMS], 0.0)

    res = cpool.tile([1, 1], F32)
    if stage >= 2:
        # gather scalars r[idx] on gpsimd (each Q7 core uses its own list)
        nc.gpsimd.ap_gather(
            gout[:], rrep[:], idx_sb[:],
            channels=128, num_elems=NELEMS, d=1, num_idxs=QCAP)
        gsum = pool.tile([128, 1], F32, tag="gsum")
        nc.vector.tensor_reduce(gsum[:], gout[:], mybir.AxisListType.X,
                                mybir.AluOpType.add)
        allred = pool.tile([128, 1], F32, tag="allred")
        nc.gpsimd.partition_all_reduce(allred[:], gsum[:], channels=128,
                                       reduce_op=bass_isa.ReduceOp.add)
        nc.scalar.mul(res[:], allred[0:1, 0:1], 0.0625)
    else:
        nc.vector.tensor_copy(res[:], r_sb[0:1, 0:1])
    nc.sync.dma_start(out[:], res[:])


_NC_CACHE = {}


def _get_nc(loop=1, stage=2):
    key = (loop, stage)
    if key not in _NC_CACHE:
        _NC_CACHE[key] = build_nc(loop, stage)
    return _NC_CACHE[key]


# shard slot s holds vocab row (s % 128) * 98 + s // 128 so that the
# device's [p, c] tiling + DRAM bounce lands r in vocab order
_S = np.arange(SH)
_ROWPERM = (_S % 128) * NCOL + (_S // 128)


def _shard_inputs(eb_input, W0, W1, W2):
    """Route indices / slice tables per core. Pure data movement."""
    idx = np.asarray(eb_input).astype(np.int64, copy=False)
    in_maps = []
    order = np.argsort(idx, kind="stable")
    sorted_idx = idx[order]
    bounds = np.searchsorted(sorted_idx, np.arange(NCORES + 1) * SH)
    for c in range(NCORES):
        lo, hi = c * SH, (c + 1) * SH
        li = (sorted_idx[bounds[c]:bounds[c + 1]] - lo).astype(np.int16)
        n = li.shape[0]
        if n > NIDX:
            raise ValueError(f"core {c} bucket {n} > NIDX {NIDX}")
        pad = np.full(NIDX, PAD_IDX, np.int16)
        pad[:n] = li
        # round-robin across the 8 Q7 cores (keeps the lists balanced),
        # then wrap each 13312-entry list into 16 partitions
        qlists = np.ascontiguousarray(pad.reshape(QCAP, 8).T)   # [8, QCAP]
        idx16 = np.ascontiguousarray(
            qlists.reshape(8, QCAP // 16, 16).transpose(0, 2, 1)
        ).reshape(128, QCAP // 16)

        def shard(W):
            ws = np.zeros((SH, D), np.float32)
            real = min(hi, V) - lo
            if real > 0:
                ws[:real] = W[lo:lo + real]
            return np.ascontiguousarray(ws[_ROWPERM])

        in_maps.append({
            "w0": shard(W0), "w1": shard(W1), "w2": shard(W2),
            "idx": idx16,
        })
    return in_maps


def _run(inputs, trace=False):
    nc = _get_nc()
    in_maps = _shard_inputs(inputs["eb_input"], inputs["W0"], inputs["W1"],
                            inputs["W2"])
    res = run_bass_kernel_spmd(nc, in_maps, core_ids=list(range(NCORES)),
                               trace=trace)
    total = np.float64(0.0)
    for r in res.results:
        total += np.float64(r["out"][0, 0])
    return np.float32(total), res


def kernel(**inputs) -> np.ndarray:
    out, _ = _run(inputs, trace=False)
    return np.asarray(out)
